# revision 1
# baseline (speedup 1.0000x reference)
"""Experts-choose MoE MLP kernel for 8 TRN2 NeuronCores — stage-skewed.

Sharding: core = 2*b + half handles batch row b and experts
[4*half, 4*half+4). Each core emits a partial out(4096,2048); host adds the
two halves per batch row.

The per-(expert, 256-token chunk) work is split into 6 stages run as a
software pipeline over the chunk list, so that within one pipeline step
every engine's work only consumes results produced in earlier steps
(keeps TensorE runs dense -> HAM stays at full clock):
  A: indirect gather of x rows (+ S-matrix build off idx/gate)
  B: PE transposes sel -> selT (D onto partitions)
  C: GEMM1 (k=D, fp32r; b1 folded in as k=1 matmul) + exact Gelu
  D: h-space gated dedup (H2T/gg/cnt matmuls) + copies
  E: GEMM2 (k=H; gated b2 as k=1 matmul) + skip-index calc + y copies
  F: accumulate-scatter (CCE add) into out; non-first duplicate rows are
     redirected to per-partition trash rows [T, T+P) of the padded output.
Output buffers are pre-zeroed by the runtime; untouched tokens stay 0.
"""

import threading

import numpy as np

import concourse.mybir as mybir
import concourse.tile as tile
from concourse import bacc
from concourse.bass import IndirectOffsetOnAxis
from concourse.bass_utils import run_bass_kernel_spmd

P = 128
B, T, D, E, C = 4, 4096, 2048, 8, 1024
H = 256
E_LOC = 4
NCB = C // P
NCP = NCB // 2
N_CORES = 8

F32 = mybir.dt.float32
F32R = mybir.dt.float32r
I32 = mybir.dt.int32
AF = mybir.ActivationFunctionType
OP = mybir.AluOpType


def build_kernel():
    nc = bacc.Bacc("TRN2", target_bir_lowering=False, debug=False)

    x = nc.dram_tensor("x", [T, D], F32R, kind="ExternalInput").ap()
    w1t = nc.dram_tensor("w1t", [E_LOC, D, H], F32R, kind="ExternalInput").ap()
    w2t = nc.dram_tensor("w2t", [E_LOC, H, D], F32R, kind="ExternalInput").ap()
    b1 = nc.dram_tensor("b1", [E_LOC, H], F32R, kind="ExternalInput").ap()
    b2 = nc.dram_tensor("b2", [D], F32R, kind="ExternalInput").ap()
    idx = nc.dram_tensor("idx", [E_LOC, P, NCB], I32, kind="ExternalInput").ap()
    gate = nc.dram_tensor("gate", [E_LOC, P, NCB], F32R, kind="ExternalInput").ap()
    ident_d = nc.dram_tensor("ident", [P, P], F32R, kind="ExternalInput").ap()
    lt_d = nc.dram_tensor("lt", [P, P], F32R, kind="ExternalInput").ap()
    ones_d = nc.dram_tensor("ones", [P, P], F32R, kind="ExternalInput").ap()
    # trash[p] = T + p : unique redirect rows for duplicate hits
    trash_d = nc.dram_tensor("trash", [P, 1], F32R, kind="ExternalInput").ap()
    out = nc.dram_tensor("out", [T, D], F32, kind="ExternalOutput").ap()

    steps = [(e, cp) for e in range(E_LOC) for cp in range(NCP)]
    NS = len(steps)

    with tile.TileContext(nc) as tc:
        with (
            tc.tile_pool(name="const", bufs=1) as const,
            tc.tile_pool(name="wts", bufs=2) as wpool,
            tc.tile_pool(name="meta", bufs=2) as mpool,
            tc.tile_pool(name="work", bufs=2) as spool,
            tc.tile_pool(name="selp", bufs=3) as selpool,
            tc.tile_pool(name="psum", bufs=2, space="PSUM") as ppool,
        ):
            ident = const.tile([P, P], F32R, tag="ident", name="ident")
            nc.sync.dma_start(out=ident, in_=ident_d)
            lt = const.tile([P, P], F32R, tag="lt", name="lt")
            nc.sync.dma_start(out=lt, in_=lt_d)
            ones = const.tile([P, P], F32R, tag="ones", name="ones")
            nc.sync.dma_start(out=ones, in_=ones_d)
            b2_row = const.tile([1, D], F32R, tag="b2row", name="b2_row")
            nc.sync.dma_start(out=b2_row, in_=b2[None, :])
            trash = const.tile([P, 1], F32R, tag="trash", name="trash")
            nc.sync.dma_start(out=trash, in_=trash_d)

            meta = {}
            wts = {}
            st = {}  # per-step pipeline state

            def load_expert(e):
                w1_sb = wpool.tile([P, D // P, H], F32R, tag="w1", name="w1_sb")
                nc.sync.dma_start(
                    out=w1_sb, in_=w1t[e].rearrange("(go gi) j -> gi go j", gi=P)
                )
                w2_sb = wpool.tile([P, H // P, D], F32R, tag="w2", name="w2_sb")
                nc.sync.dma_start(
                    out=w2_sb, in_=w2t[e].rearrange("(jo ji) o -> ji jo o", ji=P)
                )
                b1_row = wpool.tile([1, H], F32R, tag="b1", name="b1_row")
                nc.sync.dma_start(out=b1_row, in_=b1[e][None, :])
                wts[e] = (w1_sb, w2_sb, b1_row)

            def load_meta(e):
                idx_sb = mpool.tile([P, NCB], I32, tag="idx", name="idx_sb")
                nc.sync.dma_start(out=idx_sb, in_=idx[e])
                gate_sb = mpool.tile([P, NCB], F32R, tag="gate", name="gate_sb")
                nc.sync.dma_start(out=gate_sb, in_=gate[e])
                meta[e] = (idx_sb, gate_sb)

            def stage_a(s):
                e, cp = steps[s]
                if cp == 0:
                    load_meta(e)
                    load_expert(e)
                idx_sb, gate_sb = meta[e]
                sel = selpool.tile([P, 2, D], F32R, tag="sel", name="sel")
                for hh in range(2):
                    cb = cp * 2 + hh
                    nc.gpsimd.indirect_dma_start(
                        out=sel[:, hh, :],
                        out_offset=None,
                        in_=x,
                        in_offset=IndirectOffsetOnAxis(
                            ap=idx_sb[:, cb : cb + 1], axis=0
                        ),
                    )
                st[s] = {"sel": sel}

            def stage_b(s):
                e, cp = steps[s]
                idx_sb, gate_sb = meta[e]
                d = st[s]
                sel = d["sel"]
                # PE transposes of the gathered rows
                selt = spool.tile([P, D // P, 2 * P], F32R, tag="selt",
                                  name="selt")
                for hh in range(2):
                    for g4 in range(4):
                        pt = ppool.tile([P, 4 * P], F32R, tag="pt", name="pt")
                        for gg_ in range(4):
                            g = g4 * 4 + gg_
                            nc.tensor.transpose(
                                out=pt[:, gg_ * P : (gg_ + 1) * P],
                                in_=sel[:, hh, g * P : (g + 1) * P],
                                identity=ident,
                            )
                        nc.vector.tensor_copy(
                            out=selt[:, g4 * 4 : (g4 + 1) * 4,
                                     hh * P : (hh + 1) * P],
                            in_=pt.rearrange("p (g c) -> p g c", g=4),
                        )
                # selection matrices (independent of sel; only idx/gate)
                idxfs, sgls = [], []
                for hh in range(2):
                    cb = cp * 2 + hh
                    idxf = spool.tile([P, 1], F32R, tag=f"idxf{hh}",
                                      name="idxf", bufs=4)
                    nc.vector.tensor_copy(out=idxf, in_=idx_sb[:, cb : cb + 1])
                    pit = ppool.tile([P, 4 * P], F32R, tag="pt", name="pit")
                    nc.tensor.transpose(
                        out=pit[:, :P],
                        in_=idxf.to_broadcast([P, P]),
                        identity=ident,
                    )
                    idxt = spool.tile([P, P], F32, tag="idxt", name="idxt")
                    nc.vector.tensor_copy(out=idxt, in_=pit[:, :P])
                    sgl = spool.tile([P, 2, P], F32R, tag=f"sgl{hh}",
                                     name="sgl", bufs=3)
                    nc.vector.tensor_tensor(
                        out=sgl[:, 0, :],
                        in0=idxf.to_broadcast([P, P]),
                        in1=idxt,
                        op=OP.is_equal,
                    )
                    nc.vector.tensor_tensor(
                        out=sgl[:, 0, :],
                        in0=sgl[:, 0, :],
                        in1=gate_sb[:, cb : cb + 1].to_broadcast([P, P]),
                        op=OP.mult,
                    )
                    nc.vector.tensor_tensor(
                        out=sgl[:, 1, :],
                        in0=sgl[:, 0, :],
                        in1=lt,
                        op=OP.mult,
                    )
                    idxfs.append(idxf)
                    sgls.append(sgl)
                d.update(selt=selt, idxfs=idxfs, sgls=sgls)

            def stage_c(s):
                e, cp = steps[s]
                w1_sb, _, b1_row = wts[e]
                d = st[s]
                selt = d["selt"]
                hs = []
                for hh in range(2):
                    ph = ppool.tile([P, H], F32, tag="ph", name="ph")
                    for g in range(D // P):
                        nc.tensor.matmul(
                            out=ph,
                            lhsT=selt[:, g, hh * P : (hh + 1) * P],
                            rhs=w1_sb[:, g, :],
                            start=(g == 0),
                            stop=False,
                        )
                    nc.tensor.matmul(
                        out=ph, lhsT=ones[0:1, :], rhs=b1_row,
                        start=False, stop=True,
                    )
                    h_sb = spool.tile([P, H], F32R, tag=f"h{hh}", name="h_sb",
                                      bufs=2)
                    nc.scalar.activation(out=h_sb, in_=ph, func=AF.Gelu)
                    hs.append(h_sb)
                d["hs"] = hs

            def stage_d(s):
                d = st[s]
                ht2s, ggs, ph2s = [], [], []
                for hh in range(2):
                    h_sb = d["hs"][hh]
                    sgl = d["sgls"][hh]
                    # ph2 bank: [0:256]=H2T, rows[0:2] of [256:384]=gg,
                    # [384:386]=dup cnt
                    ph2 = ppool.tile([P, 4 * P], F32, tag="ph2", name="ph2")
                    for jo in range(H // P):
                        nc.tensor.matmul(
                            out=ph2[:, jo * P : (jo + 1) * P],
                            lhsT=h_sb[:, jo * P : (jo + 1) * P],
                            rhs=sgl[:, 0, :],
                            start=True,
                            stop=True,
                        )
                    nc.tensor.matmul(
                        out=ph2[0:2, 2 * P : 3 * P],
                        lhsT=ones[:, 0:2],
                        rhs=sgl[:, 0, :],
                        start=True,
                        stop=True,
                    )
                    nc.tensor.matmul(
                        out=ph2[:, 3 * P : 3 * P + 2],
                        lhsT=sgl[:, 1, :],
                        rhs=ones[:, 0:2],
                        start=True,
                        stop=True,
                    )
                    ht2 = spool.tile([P, H // P, P], F32R, tag=f"ht2{hh}",
                                     name="ht2", bufs=2)
                    for jo in range(H // P):
                        nc.vector.tensor_copy(
                            out=ht2[:, jo, :],
                            in_=ph2[:, jo * P : (jo + 1) * P],
                        )
                    gg_sb = spool.tile([1, P], F32R, tag=f"gg{hh}",
                                       name="gg_sb", bufs=3)
                    nc.vector.tensor_copy(out=gg_sb,
                                          in_=ph2[0:1, 2 * P : 3 * P])
                    ht2s.append(ht2)
                    ggs.append(gg_sb)
                    ph2s.append(ph2)
                d.update(ht2s=ht2s, ggs=ggs, ph2s=ph2s)

            def stage_e(s):
                e, cp = steps[s]
                _, w2_sb, _ = wts[e]
                d = st[s]
                y_out = spool.tile([P, 2, D], F32, tag="y", name="y_out")
                idx_skip = spool.tile([P, 2], I32, tag="idxs", name="idx_skip")
                for hh in range(2):
                    ht2 = d["ht2s"][hh]
                    gg_sb = d["ggs"][hh]
                    ph2 = d["ph2s"][hh]
                    idxf = d["idxfs"][hh]
                    # idx_skip = dup ? trash_row : idx  (m = min(cnt,1))
                    m = spool.tile([P, 1], F32, tag="m", name="m")
                    nc.vector.tensor_scalar(
                        m, ph2[:, 3 * P : 3 * P + 1], 0.0, 1.0,
                        OP.is_gt, OP.mult,
                    )
                    delta = spool.tile([P, 1], F32, tag="delta", name="delta")
                    nc.vector.tensor_scalar_mul(delta, m, float(2**24))
                    nc.vector.tensor_add(out=delta, in0=delta, in1=idxf)
                    nc.vector.tensor_copy(out=idx_skip[:, hh : hh + 1],
                                          in_=delta)

                    for oc in range(D // 512):
                        py = ppool.tile([P, 512], F32, tag="py", name="py")
                        for jo in range(H // P):
                            nc.tensor.matmul(
                                out=py,
                                lhsT=ht2[:, jo, :],
                                rhs=w2_sb[:, jo, oc * 512 : (oc + 1) * 512],
                                start=(jo == 0),
                                stop=False,
                            )
                        nc.tensor.matmul(
                            out=py,
                            lhsT=gg_sb,
                            rhs=b2_row[:, oc * 512 : (oc + 1) * 512],
                            start=False,
                            stop=True,
                        )
                        nc.scalar.copy(
                            out=y_out[:, hh, oc * 512 : (oc + 1) * 512],
                            in_=py,
                        )
                d.update(y_out=y_out, idx_skip=idx_skip)

            def stage_f(s):
                d = st.pop(s)
                for hh in range(2):
                    nc.gpsimd.indirect_dma_start(
                        out=out,
                        out_offset=IndirectOffsetOnAxis(
                            ap=d["idx_skip"][:, hh : hh + 1], axis=0
                        ),
                        in_=d["y_out"][:, hh, :],
                        in_offset=None,
                        compute_op=OP.add,
                        bounds_check=T - 1,
                        oob_is_err=False,
                    )

            stages = [stage_a, stage_b, stage_c, stage_d, stage_e, stage_f]
            for si in range(NS + len(stages) - 1):
                for k, fn in enumerate(stages):
                    s = si - k
                    if 0 <= s < NS:
                        fn(s)
    nc.compile()
    return nc


_CACHE = {}
_CACHE_LOCK = threading.Lock()


def _get_nc():
    with _CACHE_LOCK:
        if "nc" not in _CACHE:
            _CACHE["nc"] = build_kernel()
        return _CACHE["nc"]


def _make_in_maps(x, W1, b1, W2, b2, expert_indices, expert_gate):
    x = np.ascontiguousarray(x, dtype=np.float32)
    W1 = np.asarray(W1, dtype=np.float32)
    b1 = np.asarray(b1, dtype=np.float32)
    W2 = np.asarray(W2, dtype=np.float32)
    b2 = np.ascontiguousarray(b2, dtype=np.float32)
    idx = np.asarray(expert_indices, dtype=np.int32)
    gate = np.asarray(expert_gate, dtype=np.float32)

    ident = np.eye(P, dtype=np.float32)
    lt = np.triu(np.ones((P, P), dtype=np.float32), 1)
    ones = np.ones((P, P), dtype=np.float32)
    trash = (T + np.arange(P, dtype=np.float32))[:, None]

    in_maps = []
    for core in range(N_CORES):
        b, half = divmod(core, 2)
        es = slice(half * E_LOC, half * E_LOC + E_LOC)
        idx_t = np.ascontiguousarray(
            idx[b, es].reshape(E_LOC, NCB, P).transpose(0, 2, 1)
        )
        gate_t = np.ascontiguousarray(
            gate[b, es].reshape(E_LOC, NCB, P).transpose(0, 2, 1)
        )
        in_maps.append(
            {
                "x": np.ascontiguousarray(x[b]),
                "w1t": np.ascontiguousarray(W1[es].transpose(0, 2, 1)),
                "w2t": np.ascontiguousarray(W2[es].transpose(0, 2, 1)),
                "b1": np.ascontiguousarray(b1[es]),
                "b2": b2,
                "idx": idx_t,
                "gate": gate_t,
                "ident": ident,
                "lt": lt,
                "ones": ones,
                "trash": trash,
            }
        )
    return in_maps


def kernel(x, W1, b1, W2, b2, expert_indices, expert_gate, num_tokens, *,
           _trace=False, _trace_kwargs=None):
    assert int(num_tokens) == T
    nc = _get_nc()
    in_maps = _make_in_maps(x, W1, b1, W2, b2, expert_indices, expert_gate)
    res = run_bass_kernel_spmd(
        nc,
        in_maps,
        core_ids=list(range(N_CORES)),
        trace=_trace,
        **(_trace_kwargs or {}),
    )
    outs = [r["out"] for r in res.results]
    full = np.empty((B, T, D), dtype=np.float32)
    for b in range(B):
        np.add(outs[2 * b], outs[2 * b + 1], out=full[b])
    if _trace:
        kernel.last_results = res
    return full



# revision 7
# speedup vs baseline: 1.4860x; 1.4860x over previous
"""Experts-choose MoE MLP kernel for 8 TRN2 NeuronCores — bf16 rewrite.

Sharding: core = 2*b + half handles batch row b and experts
[4*half, 4*half+4). Per expert the HOST pre-deduplicates the C=1024
token picks (np.unique, gates summed over duplicates), padding the tail
with gather-row-0 / gate-0 / OOB-scatter-target slots. On device, per
expert:

  - one dma_gather(transpose=True) pulls the <=1024 unique token rows
    from bf16 x directly into the transposed [P, D/P, C] layout GEMM1
    needs as its stationary operand (no PE transposes of sel),
  - per 128-token chunk: GEMM1 (k=D, bf16, b1 folded as k=1 matmul),
    exact Gelu, 2 PE transposes h->hT, GEMM2 (k=H, bf16), gate applied
    as scale during the 4 PSUM->SBUF y copies (2 on ACT, 2 on DVE),
  - one plain indirect scatter per chunk into a PER-EXPERT bf16 output
    region (pre-dedup makes rows unique within an expert, so no
    read-modify-write accumulate is needed; pad slots point past T and
    are dropped by the bounds check).

The host combines: full[b] = sum over the 8 expert regions of rows at
each expert's unique indices, plus the rank-1 bias term
g_tot[b] (x) b2, where g_tot[t] = sum of gates routed to token t
(computable host-side from idx/gate alone).
"""

import threading

import ml_dtypes
import numpy as np

import concourse.mybir as mybir
import concourse.tile as tile
from concourse import bacc
from concourse.bass import IndirectOffsetOnAxis
from concourse.bass_utils import run_bass_kernel_spmd

P = 128
B, T, D, E, C = 4, 4096, 2048, 8, 1024
H = 256
E_LOC = 4
NCH = C // P   # 8 chunks per expert
KG = D // P    # 16 contraction groups for GEMM1
JG = H // P    # 2 contraction groups for GEMM2
N_CORES = 8
PAD_IDX = 1 << 22  # scatter target for pad slots; > T-1 so bounds check drops it

BF16 = mybir.dt.bfloat16
F32 = mybir.dt.float32
I32 = mybir.dt.int32
I16 = mybir.dt.int16
AF = mybir.ActivationFunctionType
OP = mybir.AluOpType

NPBF16 = ml_dtypes.bfloat16


def build_kernel():
    nc = bacc.Bacc("TRN2", target_bir_lowering=False, debug=False)

    x = nc.dram_tensor("x", [T, D], BF16, kind="ExternalInput").ap()
    w1t = nc.dram_tensor("w1t", [E_LOC, P, KG, H], BF16, kind="ExternalInput").ap()
    w2t = nc.dram_tensor("w2t", [E_LOC, P, JG, D], BF16, kind="ExternalInput").ap()
    b1r = nc.dram_tensor("b1r", [E_LOC, 1, H], BF16, kind="ExternalInput").ap()
    ones_d = nc.dram_tensor("ones", [1, P], BF16, kind="ExternalInput").ap()
    ident_d = nc.dram_tensor("ident", [P, P], BF16, kind="ExternalInput").ap()
    gidx_d = nc.dram_tensor("gidx", [E_LOC, P, C // 16], I16,
                            kind="ExternalInput").ap()
    sidx_d = nc.dram_tensor("sidx", [E_LOC, P, NCH], I32,
                            kind="ExternalInput").ap()
    gsc_d = nc.dram_tensor("gsc", [E_LOC, P, NCH], F32, kind="ExternalInput").ap()
    outs = [
        nc.dram_tensor(f"out{e}", [T, D], BF16, kind="ExternalOutput").ap()
        for e in range(E_LOC)
    ]

    with tile.TileContext(nc) as tc:
        with (
            tc.tile_pool(name="const", bufs=1) as const,
            tc.tile_pool(name="sel", bufs=2) as selpool,
            tc.tile_pool(name="work", bufs=2) as wk,
            tc.tile_pool(name="ytile", bufs=3) as ypool,
            tc.tile_pool(name="ph", bufs=2, space="PSUM") as php,
            tc.tile_pool(name="pt", bufs=2, space="PSUM") as ptp,
            tc.tile_pool(name="py", bufs=1, space="PSUM") as pyp,
        ):
            ident = const.tile([P, P], BF16, tag="ident", name="ident")
            nc.sync.dma_start(out=ident, in_=ident_d)
            ones = const.tile([1, P], BF16, tag="ones", name="ones")
            nc.sync.dma_start(out=ones, in_=ones_d)

            w1_sb, w2_sb, b1_sb, gi_sb, si_sb, gs_sb = {}, {}, {}, {}, {}, {}
            for e in range(E_LOC):
                w1_sb[e] = const.tile([P, KG, H], BF16, tag=f"w1_{e}", name="w1sb")
                nc.sync.dma_start(out=w1_sb[e], in_=w1t[e])
                w2_sb[e] = const.tile([P, JG, D], BF16, tag=f"w2_{e}", name="w2sb")
                nc.sync.dma_start(out=w2_sb[e], in_=w2t[e])
                b1_sb[e] = const.tile([1, H], BF16, tag=f"b1_{e}", name="b1sb")
                nc.sync.dma_start(out=b1_sb[e], in_=b1r[e])
                gi_sb[e] = const.tile([P, C // 16], I16, tag=f"gi_{e}", name="gisb")
                nc.sync.dma_start(out=gi_sb[e], in_=gidx_d[e])
                si_sb[e] = const.tile([P, NCH], I32, tag=f"si_{e}", name="sisb")
                nc.sync.dma_start(out=si_sb[e], in_=sidx_d[e])
                gs_sb[e] = const.tile([P, NCH], F32, tag=f"gs_{e}", name="gssb")
                nc.sync.dma_start(out=gs_sb[e], in_=gsc_d[e])

            selt = {}
            CH = C // 2  # 512-idx gather halves (ring capacity is 1024 descs)

            def gather(e):
                pair = []
                for hh in range(2):
                    stile = selpool.tile([P, KG, CH], BF16, tag=f"selt{hh}",
                                         name="selt")
                    nc.gpsimd.dma_gather(stile, x, gi_sb[e][:, hh * (CH // 16):
                                                            (hh + 1) * (CH // 16)],
                                         CH, CH, D, transpose=True)
                    pair.append(stile)
                selt[e] = pair

            gather(0)
            for e in range(E_LOC):
                if e + 1 < E_LOC:
                    gather(e + 1)
                spair = selt.pop(e)
                for c in range(NCH):
                    st = spair[c // 4]
                    cs = (c % 4) * P
                    ph = php.tile([P, H], F32, tag="ph", name="ph")
                    for g in range(KG):
                        nc.tensor.matmul(
                            out=ph,
                            lhsT=st[:, g, cs:cs + P],
                            rhs=w1_sb[e][:, g, :],
                            start=(g == 0),
                            stop=False,
                        )
                    nc.tensor.matmul(out=ph, lhsT=ones, rhs=b1_sb[e],
                                     start=False, stop=True)
                    h2 = wk.tile([P, H], BF16, tag="h2", name="h2")
                    nc.scalar.activation(out=h2, in_=ph, func=AF.Gelu)
                    pt = ptp.tile([P, H], BF16, tag="pt", name="pt")
                    for j in range(JG):
                        nc.tensor.transpose(
                            out=pt[:, j * P:(j + 1) * P],
                            in_=h2[:, j * P:(j + 1) * P],
                            identity=ident,
                        )
                    hT = wk.tile([P, H], BF16, tag="hT", name="hT")
                    nc.vector.tensor_copy(out=hT, in_=pt)
                    py = pyp.tile([P, D], F32, tag="py", name="py")
                    for oc in range(4):
                        ocs = oc * 512
                        for jo in range(JG):
                            nc.tensor.matmul(
                                out=py[:, ocs:ocs + 512],
                                lhsT=hT[:, jo * P:(jo + 1) * P],
                                rhs=w2_sb[e][:, jo, ocs:ocs + 512],
                                start=(jo == 0),
                                stop=(jo == JG - 1),
                            )
                    y = ypool.tile([P, D], BF16, tag="y", name="y")
                    gcol = gs_sb[e][:, c:c + 1]
                    nc.scalar.activation(out=y[:, 0:512], in_=py[:, 0:512],
                                         func=AF.Copy, scale=gcol)
                    nc.scalar.activation(out=y[:, 512:1024], in_=py[:, 512:1024],
                                         func=AF.Copy, scale=gcol)
                    nc.vector.tensor_tensor(
                        out=y[:, 1024:1536], in0=py[:, 1024:1536],
                        in1=gcol.to_broadcast([P, 512]), op=OP.mult)
                    nc.vector.tensor_tensor(
                        out=y[:, 1536:2048], in0=py[:, 1536:2048],
                        in1=gcol.to_broadcast([P, 512]), op=OP.mult)
                    nc.gpsimd.indirect_dma_start(
                        out=outs[e],
                        out_offset=IndirectOffsetOnAxis(
                            ap=si_sb[e][:, c:c + 1], axis=0),
                        in_=y,
                        in_offset=None,
                        bounds_check=T - 1,
                        oob_is_err=False,
                    )
    nc.compile()
    return nc


_CACHE = {}
_CACHE_LOCK = threading.Lock()


def _get_nc():
    with _CACHE_LOCK:
        if "nc" not in _CACHE:
            _CACHE["nc"] = build_kernel()
        return _CACHE["nc"]


def _prep(x, W1, b1, W2, b2, expert_indices, expert_gate):
    """Host-side shard + pre-dedup. Returns (in_maps, combine_meta)."""
    x = np.asarray(x, dtype=np.float32)
    W1 = np.asarray(W1, dtype=np.float32)
    b1 = np.asarray(b1, dtype=np.float32)
    W2 = np.asarray(W2, dtype=np.float32)
    b2 = np.asarray(b2, dtype=np.float32)
    idx = np.asarray(expert_indices, dtype=np.int64)
    gate = np.asarray(expert_gate, dtype=np.float64)

    x_bf = [np.ascontiguousarray(x[b]).astype(NPBF16) for b in range(B)]
    # w1t[e, dlo, g, h] = W1[e][h, g*P+dlo];  w2t[e, hlo, jo, d] = W2[e][d, jo*P+hlo]
    w1t_all = np.ascontiguousarray(
        W1.transpose(0, 2, 1).reshape(E, KG, P, H).transpose(0, 2, 1, 3)
    ).astype(NPBF16)
    w2t_all = np.ascontiguousarray(
        W2.transpose(0, 2, 1).reshape(E, JG, P, D).transpose(0, 2, 1, 3)
    ).astype(NPBF16)
    b1_bf = b1.astype(NPBF16)

    ident = np.eye(P, dtype=np.float32).astype(NPBF16)
    ones = np.ones((1, P), dtype=np.float32).astype(NPBF16)

    # g_tot[b, t] = total gate routed to token t of batch row b (for the
    # host-side rank-1 b2 term).
    g_tot = np.zeros((B, T), dtype=np.float64)
    for b in range(B):
        np.add.at(g_tot[b], idx[b].ravel(), gate[b].ravel())

    in_maps = []
    uniqs = []  # per core: list of per-expert (uniq_indices,) for combine
    for core in range(N_CORES):
        b, half = divmod(core, 2)
        es = half * E_LOC
        gidx_m = np.zeros((E_LOC, P, C // 16), dtype=np.int16)
        sidx_m = np.full((E_LOC, P, NCH), PAD_IDX, dtype=np.int32)
        gsc_m = np.zeros((E_LOC, P, NCH), dtype=np.float32)
        core_uniqs = []
        for el in range(E_LOC):
            tok = idx[b, es + el]
            g = gate[b, es + el]
            uniq, inv = np.unique(tok, return_inverse=True)
            gsum = np.zeros(len(uniq), dtype=np.float64)
            np.add.at(gsum, inv, g)
            n_u = len(uniq)
            garr = np.zeros(C, dtype=np.float32)
            garr[:n_u] = gsum.astype(np.float32)
            iarr = np.zeros(C, dtype=np.int16)
            iarr[:n_u] = uniq.astype(np.int16)
            sarr = np.full(C, PAD_IDX, dtype=np.int32)
            sarr[:n_u] = uniq.astype(np.int32)
            # wrapped in 16 partitions, replicated across the 8 Q7 cores
            # (each core reads its own 16-partition stripe on HW)
            gidx_m[el][:] = np.tile(iarr.reshape(C // 16, 16).T, (P // 16, 1))
            sidx_m[el] = sarr.reshape(NCH, P).T
            gsc_m[el] = garr.reshape(NCH, P).T
            core_uniqs.append(uniq)
        uniqs.append(core_uniqs)
        in_maps.append({
            "x": x_bf[b],
            "w1t": np.ascontiguousarray(w1t_all[es:es + E_LOC]),
            "w2t": np.ascontiguousarray(w2t_all[es:es + E_LOC]),
            "b1r": np.ascontiguousarray(b1_bf[es:es + E_LOC][:, None, :]),
            "ones": ones,
            "ident": ident,
            "gidx": gidx_m,
            "sidx": sidx_m,
            "gsc": gsc_m,
        })
    return in_maps, (uniqs, g_tot, b2)


def _combine(results, meta):
    uniqs, g_tot, b2 = meta
    full = np.empty((B, T, D), dtype=np.float32)
    for b in range(B):
        acc = g_tot[b].astype(np.float32)[:, None] * b2[None, :]
        for half in range(2):
            core = 2 * b + half
            r = results[core]
            for el in range(E_LOC):
                u = uniqs[core][el]
                acc[u] += r[f"out{el}"][u].astype(np.float32)
        full[b] = acc
    return full


def kernel(x, W1, b1, W2, b2, expert_indices, expert_gate, num_tokens, *,
           _trace=False, _trace_kwargs=None):
    assert int(num_tokens) == T
    nc = _get_nc()
    in_maps, meta = _prep(x, W1, b1, W2, b2, expert_indices, expert_gate)
    res = run_bass_kernel_spmd(
        nc,
        in_maps,
        core_ids=list(range(N_CORES)),
        trace=_trace,
        **(_trace_kwargs or {}),
    )
    full = _combine(res.results, meta)
    if _trace:
        kernel.last_results = res
    return full


# revision 10
# speedup vs baseline: 1.7946x; 1.2077x over previous
"""Experts-choose MoE MLP kernel for 8 TRN2 NeuronCores — bf16 rewrite.

Sharding: core = 2*b + half handles batch row b and experts
[4*half, 4*half+4). Per expert the HOST pre-deduplicates the C=1024
token picks (np.unique, gates summed over duplicates), padding the tail
with gather-row-0 / gate-0 / OOB-scatter-target slots. On device, per
expert:

  - one dma_gather(transpose=True) pulls the <=1024 unique token rows
    from bf16 x directly into the transposed [P, D/P, C] layout GEMM1
    needs as its stationary operand (no PE transposes of sel),
  - per 128-token chunk: GEMM1 (k=D, bf16, b1 folded as k=1 matmul),
    exact Gelu, 2 PE transposes h->hT, GEMM2 (k=H, bf16), gate applied
    as scale during the 4 PSUM->SBUF y copies (2 on ACT, 2 on DVE),
  - one plain indirect scatter per chunk into a PER-EXPERT bf16 output
    region (pre-dedup makes rows unique within an expert, so no
    read-modify-write accumulate is needed; pad slots point past T and
    are dropped by the bounds check).

The host combines: full[b] = sum over the 8 expert regions of rows at
each expert's unique indices, plus the rank-1 bias term
g_tot[b] (x) b2, where g_tot[t] = sum of gates routed to token t
(computable host-side from idx/gate alone).
"""

import threading

import ml_dtypes
import numpy as np

import concourse.mybir as mybir
import concourse.tile as tile
from concourse import bacc
from concourse.bass import IndirectOffsetOnAxis
from concourse.bass_utils import run_bass_kernel_spmd

P = 128
B, T, D, E, C = 4, 4096, 2048, 8, 1024
H = 256
E_LOC = 4
NCH = C // P   # 8 chunks per expert
KG = D // P    # 16 contraction groups for GEMM1
JG = H // P    # 2 contraction groups for GEMM2
N_CORES = 8
PAD_IDX = 1 << 22  # scatter target for pad slots; > T-1 so bounds check drops it

BF16 = mybir.dt.bfloat16
F32 = mybir.dt.float32
I32 = mybir.dt.int32
I16 = mybir.dt.int16
AF = mybir.ActivationFunctionType
OP = mybir.AluOpType

NPBF16 = ml_dtypes.bfloat16


def build_kernel():
    nc = bacc.Bacc("TRN2", target_bir_lowering=False, debug=False)

    x = nc.dram_tensor("x", [T, D], BF16, kind="ExternalInput").ap()
    w1t = nc.dram_tensor("w1t", [E_LOC, P, KG, H], BF16, kind="ExternalInput").ap()
    w2t = nc.dram_tensor("w2t", [E_LOC, P, JG, D], BF16, kind="ExternalInput").ap()
    b1r = nc.dram_tensor("b1r", [E_LOC, 1, H], BF16, kind="ExternalInput").ap()
    ones_d = nc.dram_tensor("ones", [1, P], BF16, kind="ExternalInput").ap()
    ident_d = nc.dram_tensor("ident", [P, P], BF16, kind="ExternalInput").ap()
    gidx_d = nc.dram_tensor("gidx", [E_LOC, P, C // 16], I16,
                            kind="ExternalInput").ap()
    sidx_d = nc.dram_tensor("sidx", [E_LOC, P, NCH], I32,
                            kind="ExternalInput").ap()
    gsc_d = nc.dram_tensor("gsc", [E_LOC, P, NCH], F32, kind="ExternalInput").ap()
    outs = [
        nc.dram_tensor(f"out{e}", [T, D], BF16, kind="ExternalOutput").ap()
        for e in range(E_LOC)
    ]

    with tile.TileContext(nc) as tc:
        with (
            tc.tile_pool(name="const", bufs=1) as const,
            tc.tile_pool(name="sel", bufs=2) as selpool,
            tc.tile_pool(name="work", bufs=2) as wk,
            tc.tile_pool(name="ytile", bufs=3) as ypool,
            tc.tile_pool(name="ph", bufs=2, space="PSUM") as php,
            tc.tile_pool(name="pt", bufs=2, space="PSUM") as ptp,
            tc.tile_pool(name="py", bufs=2, space="PSUM") as pyp,
        ):
            ident = const.tile([P, P], BF16, tag="ident", name="ident")
            nc.sync.dma_start(out=ident, in_=ident_d)
            ones = const.tile([1, P], BF16, tag="ones", name="ones")
            nc.sync.dma_start(out=ones, in_=ones_d)

            w1_sb, w2_sb, b1_sb, gi_sb, si_sb, gs_sb = {}, {}, {}, {}, {}, {}

            def load_expert(e):
                gi_sb[e] = const.tile([P, C // 16], I16, tag=f"gi_{e}", name="gisb")
                nc.sync.dma_start(out=gi_sb[e], in_=gidx_d[e])
                w1_sb[e] = const.tile([P, KG, H], BF16, tag=f"w1_{e}", name="w1sb")
                nc.sync.dma_start(out=w1_sb[e], in_=w1t[e])
                w2_sb[e] = const.tile([P, JG, D], BF16, tag=f"w2_{e}", name="w2sb")
                nc.sync.dma_start(out=w2_sb[e], in_=w2t[e])
                b1_sb[e] = const.tile([1, H], BF16, tag=f"b1_{e}", name="b1sb")
                nc.sync.dma_start(out=b1_sb[e], in_=b1r[e])
                si_sb[e] = const.tile([P, NCH], I32, tag=f"si_{e}", name="sisb")
                nc.sync.dma_start(out=si_sb[e], in_=sidx_d[e])
                gs_sb[e] = const.tile([P, NCH], F32, tag=f"gs_{e}", name="gssb")
                nc.sync.dma_start(out=gs_sb[e], in_=gsc_d[e])

            selt = {}
            CH = C // 2  # 512-idx gather halves (ring capacity is 1024 descs)

            def gather(e):
                pair = []
                for hh in range(2):
                    stile = selpool.tile([P, KG, CH], BF16, tag=f"selt{hh}",
                                         name="selt")
                    nc.gpsimd.dma_gather(stile, x, gi_sb[e][:, hh * (CH // 16):
                                                            (hh + 1) * (CH // 16)],
                                         CH, CH, D, transpose=True)
                    pair.append(stile)
                selt[e] = pair

            NS = E_LOC * NCH
            st_state = {}

            def stage_g1(s):
                """GEMM1 (+b1) for global chunk s."""
                e, c = divmod(s, NCH)
                if c == 0 and e + 1 < E_LOC:
                    gather(e + 1)
                st = selt[e][c // 4]
                cs = (c % 4) * P
                ph = php.tile([P, H], F32, tag="ph", name="ph")
                for g in range(KG):
                    nc.tensor.matmul(
                        out=ph,
                        lhsT=st[:, g, cs:cs + P],
                        rhs=w1_sb[e][:, g, :],
                        start=(g == 0),
                        stop=False,
                    )
                nc.tensor.matmul(out=ph, lhsT=ones, rhs=b1_sb[e],
                                 start=False, stop=True)
                h2 = wk.tile([P, H], BF16, tag="h2", name="h2")
                nc.scalar.activation(out=h2, in_=ph, func=AF.Gelu)
                st_state[s] = h2

            def stage_g2(s):
                """Transpose + GEMM2 halves + gated copies + scatter, chunk s."""
                e, c = divmod(s, NCH)
                h2 = st_state.pop(s)
                pt = ptp.tile([P, H], BF16, tag="pt", name="pt")
                for j in range(JG):
                    nc.tensor.transpose(
                        out=pt[:, j * P:(j + 1) * P],
                        in_=h2[:, j * P:(j + 1) * P],
                        identity=ident,
                    )
                hT = wk.tile([P, H], BF16, tag="hT", name="hT")
                nc.vector.tensor_copy(out=hT, in_=pt)
                y = ypool.tile([P, D], BF16, tag="y", name="y")
                gcol = gs_sb[e][:, c:c + 1]
                for half in range(2):
                    hs = half * 1024
                    py = pyp.tile([P, 1024], F32, tag="py", name="py")
                    for jo in range(JG):
                        for oc in range(2):
                            nc.tensor.matmul(
                                out=py[:, oc * 512:(oc + 1) * 512],
                                lhsT=hT[:, jo * P:(jo + 1) * P],
                                rhs=w2_sb[e][:, jo, hs + oc * 512:
                                             hs + (oc + 1) * 512],
                                start=(jo == 0),
                                stop=(jo == JG - 1),
                                skip_group_check=True,
                            )
                    nc.scalar.activation(out=y[:, hs:hs + 512],
                                         in_=py[:, 0:512],
                                         func=AF.Copy, scale=gcol)
                    nc.vector.tensor_tensor(
                        out=y[:, hs + 512:hs + 1024], in0=py[:, 512:1024],
                        in1=gcol.to_broadcast([P, 512]), op=OP.mult)
                nc.gpsimd.indirect_dma_start(
                    out=outs[e],
                    out_offset=IndirectOffsetOnAxis(
                        ap=si_sb[e][:, c:c + 1], axis=0),
                    in_=y,
                    in_offset=None,
                    bounds_check=T - 1,
                    oob_is_err=False,
                )
                if (s + 1) % NCH == 0:
                    del selt[e]

            load_expert(0)
            gather(0)
            for e in range(1, E_LOC):
                load_expert(e)
            stage_g1(0)
            for s in range(1, NS):
                stage_g1(s)
                stage_g2(s - 1)
            stage_g2(NS - 1)
    nc.compile()
    return nc


_CACHE = {}
_CACHE_LOCK = threading.Lock()


def _get_nc():
    with _CACHE_LOCK:
        if "nc" not in _CACHE:
            _CACHE["nc"] = build_kernel()
        return _CACHE["nc"]


def _prep(x, W1, b1, W2, b2, expert_indices, expert_gate):
    """Host-side shard + pre-dedup. Returns (in_maps, combine_meta)."""
    x = np.asarray(x, dtype=np.float32)
    W1 = np.asarray(W1, dtype=np.float32)
    b1 = np.asarray(b1, dtype=np.float32)
    W2 = np.asarray(W2, dtype=np.float32)
    b2 = np.asarray(b2, dtype=np.float32)
    idx = np.asarray(expert_indices, dtype=np.int64)
    gate = np.asarray(expert_gate, dtype=np.float64)

    x_bf = [np.ascontiguousarray(x[b]).astype(NPBF16) for b in range(B)]
    # w1t[e, dlo, g, h] = W1[e][h, g*P+dlo];  w2t[e, hlo, jo, d] = W2[e][d, jo*P+hlo]
    w1t_all = np.ascontiguousarray(
        W1.transpose(0, 2, 1).reshape(E, KG, P, H).transpose(0, 2, 1, 3)
    ).astype(NPBF16)
    w2t_all = np.ascontiguousarray(
        W2.transpose(0, 2, 1).reshape(E, JG, P, D).transpose(0, 2, 1, 3)
    ).astype(NPBF16)
    b1_bf = b1.astype(NPBF16)

    ident = np.eye(P, dtype=np.float32).astype(NPBF16)
    ones = np.ones((1, P), dtype=np.float32).astype(NPBF16)

    # g_tot[b, t] = total gate routed to token t of batch row b (for the
    # host-side rank-1 b2 term).
    g_tot = np.zeros((B, T), dtype=np.float64)
    for b in range(B):
        np.add.at(g_tot[b], idx[b].ravel(), gate[b].ravel())

    in_maps = []
    uniqs = []  # per core: list of per-expert (uniq_indices,) for combine
    for core in range(N_CORES):
        b, half = divmod(core, 2)
        es = half * E_LOC
        gidx_m = np.zeros((E_LOC, P, C // 16), dtype=np.int16)
        sidx_m = np.full((E_LOC, P, NCH), PAD_IDX, dtype=np.int32)
        gsc_m = np.zeros((E_LOC, P, NCH), dtype=np.float32)
        core_uniqs = []
        for el in range(E_LOC):
            tok = idx[b, es + el]
            g = gate[b, es + el]
            uniq, inv = np.unique(tok, return_inverse=True)
            gsum = np.zeros(len(uniq), dtype=np.float64)
            np.add.at(gsum, inv, g)
            n_u = len(uniq)
            garr = np.zeros(C, dtype=np.float32)
            garr[:n_u] = gsum.astype(np.float32)
            iarr = np.zeros(C, dtype=np.int16)
            iarr[:n_u] = uniq.astype(np.int16)
            sarr = np.full(C, PAD_IDX, dtype=np.int32)
            sarr[:n_u] = uniq.astype(np.int32)
            # wrapped in 16 partitions, replicated across the 8 Q7 cores
            # (each core reads its own 16-partition stripe on HW)
            gidx_m[el][:] = np.tile(iarr.reshape(C // 16, 16).T, (P // 16, 1))
            sidx_m[el] = sarr.reshape(NCH, P).T
            gsc_m[el] = garr.reshape(NCH, P).T
            core_uniqs.append(uniq)
        uniqs.append(core_uniqs)
        in_maps.append({
            "x": x_bf[b],
            "w1t": np.ascontiguousarray(w1t_all[es:es + E_LOC]),
            "w2t": np.ascontiguousarray(w2t_all[es:es + E_LOC]),
            "b1r": np.ascontiguousarray(b1_bf[es:es + E_LOC][:, None, :]),
            "ones": ones,
            "ident": ident,
            "gidx": gidx_m,
            "sidx": sidx_m,
            "gsc": gsc_m,
        })
    return in_maps, (uniqs, g_tot, b2)


def _combine(results, meta):
    uniqs, g_tot, b2 = meta
    full = np.empty((B, T, D), dtype=np.float32)
    for b in range(B):
        acc = g_tot[b].astype(np.float32)[:, None] * b2[None, :]
        for half in range(2):
            core = 2 * b + half
            r = results[core]
            for el in range(E_LOC):
                u = uniqs[core][el]
                acc[u] += r[f"out{el}"][u].astype(np.float32)
        full[b] = acc
    return full


def kernel(x, W1, b1, W2, b2, expert_indices, expert_gate, num_tokens, *,
           _trace=False, _trace_kwargs=None):
    assert int(num_tokens) == T
    nc = _get_nc()
    in_maps, meta = _prep(x, W1, b1, W2, b2, expert_indices, expert_gate)
    res = run_bass_kernel_spmd(
        nc,
        in_maps,
        core_ids=list(range(N_CORES)),
        trace=_trace,
        **(_trace_kwargs or {}),
    )
    full = _combine(res.results, meta)
    if _trace:
        kernel.last_results = res
    return full
